# revision 23
# baseline (speedup 1.0000x reference)
"""Trainium2 Bass kernel for nn_CANLayer (gnn_message_passing) — v6.

Math: softmax over a singleton axis makes attention weights identically 1.0,
so each conv is a plain sparse matmul:
    out = sigmoid(A_d @ x @ Wd + A_u @ x @ Wu + (1+eps) x @ Wi); out *= elu(out @ a)

v3: gathers via the Ant dma_gather ucode (int16 indices); x is zero-padded to
256B rows (dma_gather elem constraint); indices split into 4 row segments of
25000 (int16 range); messages form 4 segment-major streams ordered by bin,
chunks of 128 slots, mid-stream pads use idx=0/val=0. A bin's messages span a
chunk range per segment stream; straddle chunks are matmul'd once per bin.

v4: 4 SWDGE queues — dma_gather queue_num=g pins segment stream g's
descriptor generation to Q7 core pair g, 4-way parallel (the ucode gates on
cpu_id/2 == queue_num; 3.1x on a gather microbench).

v5: bins narrowed to 128-target subwindows (4 bins per lap per 512 epilogue
window); per-subwindow PSUM [C,128] accumulators are copied into the [C,512]
ys tile, keeping the fused 512-wide epilogue; 16-bit path moved to bf16.

v6: DVE was still the wall (is_equal selection-matrix gen has a ~330-cycle
per-instruction floor). The [128,128] selection matrices (val folded in) are
now PRECOMPUTED on host and streamed from DRAM in STB-visit blocks (~88 MB
per core — HBM had headroom), so the scatter path on-device is matmul-only.

v7: epilogue rewritten to stay in ONE ACT table set (sigmoid/exp alternation
cost 2x ~2.7us table reloads per window): sigmoid(r) = 0.5*tanh(r/2)+0.5
with the affine folded into the att matmul constant and the final gate
multiply; elu(g) = relu(g) + exp(-relu(-g)) - 1 via Relu/Exp scale/bias
folding. Gather calls grown past the 64-descriptor single-packet limit with
single_packet=False (GC=21 chunks/call, 4 bufs), per-queue idx tiles so each
queue's first gather starts immediately, and the st stream double-buffered
48 visits/block 2 blocks ahead. Final: ~591 us (baseline 2327 us).
"""
import numpy as np
import ml_dtypes
from contextlib import ExitStack

import concourse.bacc as bacc
import concourse.bass as bass
import concourse.mybir as mybir
import concourse.tile as tile
from concourse.bass_utils import run_bass_kernel_spmd


def _install_ntff_shim():
    """Best-effort: some runtimes lack antenv.axon_hooks, which
    run_bass_kernel_spmd(trace=True) imports under axon."""
    import sys, types
    try:
        import antenv.axon_hooks  # noqa: F401
        return
    except ImportError:
        pass
    try:
        from trn_agent_boot.trn_boot import _ntff_profile_via_ctypes
        hook = _ntff_profile_via_ctypes('/opt/axon/libaxon_pjrt.so')
        import antenv
        mod = types.ModuleType('antenv.axon_hooks')
        mod.get_axon_ntff_profile_hook = lambda: hook
        mod.set_axon_ntff_profile_hook = lambda h: None
        antenv.axon_hooks = mod
        sys.modules['antenv.axon_hooks'] = mod
    except Exception:
        pass


_install_ntff_shim()

N = 100000
C = 64
NCORES = 8
TPC = 12500
WIN = 512                      # epilogue window
SWIN = 128                     # scatter subwindow (st width)
NW = (TPC + WIN - 1) // WIN    # 25
NSEG = 25000                   # int16 index range per gather segment
GC = 21                        # chunks per dma_gather call (2688 idxs; multi-packet)
STB = 24                       # visits per streamed st block
EPS = 1e-5
ANCHOR = 4                     # drift anchor every ANCHOR bins

LAST_EXEC_NS = None

_frontend_cache = {}

f32 = mybir.dt.float32
bf16 = mybir.dt.bfloat16
i16 = mybir.dt.int16
np_bf16 = ml_dtypes.bfloat16


def _bins():
    """Stream-ordered bin list: (w, L, sw, t0, swn) with t0 core-relative."""
    out = []
    for w in range(NW):
        wn = min(WIN, TPC - w * WIN)
        nsw = (wn + SWIN - 1) // SWIN
        for L in range(2):
            for sw in range(nsw):
                t0 = w * WIN + sw * SWIN
                swn = min(SWIN, wn - sw * SWIN)
                out.append((w, L, sw, t0, swn))
    return out


BINS = _bins()
NB = len(BINS)


def _preprocess(indices, values):
    """Per core: tgt-sorted stream (src, val, tl) for one Laplacian."""
    tgt = np.asarray(indices[0], np.int64)
    src = np.asarray(indices[1], np.int64)
    val = np.asarray(values, np.float32)
    out = []
    for k in range(NCORES):
        base = k * TPC
        sel = (tgt >= base) & (tgt < base + TPC)
        tl = tgt[sel] - base
        s = src[sel]
        v = val[sel]
        order = np.argsort(tl, kind="stable")
        out.append((s[order], v[order], tl[order]))
    return out


def _build_program(SCH, GBASE, NCH, VIS, NV):
    """SCH[g]=chunks per segment stream; VIS[j]=[(g, lo, hi)] visit ranges
    (global chunk ids) for bin j (BINS order); NV total visit columns."""
    nc = bacc.Bacc("TRN2", target_bir_lowering=False, debug=False,
                   num_swdge_queues=4)

    NSTB = (NV + STB - 1) // STB
    xp = nc.dram_tensor("xp", [N, 128], bf16, kind="ExternalInput")
    xT = nc.dram_tensor("xT", [C, TPC], bf16, kind="ExternalInput")
    idx_d = nc.dram_tensor("idx", [128, NCH * 8], i16, kind="ExternalInput")
    st_d = nc.dram_tensor("st", [128, NV * SWIN], bf16, kind="ExternalInput")
    wts_d = nc.dram_tensor("wts", [C, 3 * C], bf16, kind="ExternalInput")  # Wd|Wu|Wi'
    att_d = nc.dram_tensor("att", [C, 1], f32, kind="ExternalInput")
    cb_d = nc.dram_tensor("cb", [1, 2], f32, kind="ExternalInput")  # [catt, -catt]
    ident_d = nc.dram_tensor("ident", [128, 128], f32, kind="ExternalInput")
    out_d = nc.dram_tensor("out", [TPC, C], f32, kind="ExternalOutput")

    with tile.TileContext(nc) as tc, ExitStack() as _stk:
        if True:
            constp = _stk.enter_context(tc.tile_pool(name="const", bufs=1))
            metap = _stk.enter_context(tc.tile_pool(name="meta", bufs=1))
            mp0 = _stk.enter_context(tc.tile_pool(name="m0", bufs=4))
            mp1 = _stk.enter_context(tc.tile_pool(name="m1", bufs=4))
            mp2 = _stk.enter_context(tc.tile_pool(name="m2", bufs=4))
            mp3 = _stk.enter_context(tc.tile_pool(name="m3", bufs=4))
            stbp = _stk.enter_context(tc.tile_pool(name="stb", bufs=8))
            ysbp = _stk.enter_context(tc.tile_pool(name="ysb", bufs=4))
            xtp = _stk.enter_context(tc.tile_pool(name="xt", bufs=2))
            ssbp = _stk.enter_context(tc.tile_pool(name="ssb", bufs=2))
            epool = _stk.enter_context(tc.tile_pool(name="ep", bufs=8))
            psp = _stk.enter_context(tc.tile_pool(name="ps", bufs=3, space="PSUM"))
            rpp = _stk.enter_context(tc.tile_pool(name="rp", bufs=2, space="PSUM"))
            gpp = _stk.enter_context(tc.tile_pool(name="gp", bufs=1, space="PSUM"))
            tpp = _stk.enter_context(tc.tile_pool(name="tp", bufs=2, space="PSUM"))
            mpools = [mp0, mp1, mp2, mp3]
            ident_t = constp.tile([128, 128], f32)
            nc.sync.dma_start(out=ident_t[:], in_=ident_d[:])
            wts_t = constp.tile([C, 3 * C], bf16)
            nc.sync.dma_start(out=wts_t[:], in_=wts_d[:])
            att_t = constp.tile([C, 1], f32)
            nc.sync.dma_start(out=att_t[:], in_=att_d[:])
            cb_t = constp.tile([1, 2], f32)
            nc.sync.dma_start(out=cb_t[:], in_=cb_d[:])

            idx_ts = []
            for g in range(4):
                tg = metap.tile([128, SCH[g] * 8], i16, name=f"idx_t{g}")
                nc.sync.dma_start(
                    out=tg[:],
                    in_=idx_d[:, GBASE[g] * 8:(GBASE[g] + SCH[g]) * 8],
                )
                idx_ts.append(tg)

            # call schedule per segment stream: groups of GC chunks
            calls = []            # [g] -> list of (startchunk_local, nchunks)
            for g in range(4):
                cl = []
                a = 0
                while a < SCH[g]:
                    cl.append((a, min(GC, SCH[g] - a)))
                    a += min(GC, SCH[g] - a)
                calls.append(cl)
            call_tiles = [dict() for _ in range(4)]
            issued = [0, 0, 0, 0]

            def issue_call(g):
                ci = issued[g]
                a, cnt = calls[g][ci]
                t = mpools[g].tile([128, GC, 128], bf16, tag=f"m{g}")
                ni = cnt * 128
                io = a * 8
                nc.gpsimd.dma_gather(
                    t[:, :cnt, :],
                    xp[g * NSEG:(g + 1) * NSEG, :],
                    idx_ts[g][:, io:io + cnt * 8],
                    ni, ni, 128, elem_step=128, queue_num=g,
                    single_packet=False,
                )
                call_tiles[g][ci] = t
                issued[g] = ci + 1

            for g in range(4):
                issue_call(g)
                if len(calls[g]) > 1:
                    issue_call(g)

            # st stream: blocks of STB visit columns, double-buffered
            st_tiles = {}
            st_issued = [0]

            def issue_st():
                b = st_issued[0]
                t = stbp.tile([128, STB * SWIN], bf16, tag="stb")
                n = min(STB, NV - b * STB)
                nc.sync.dma_start(
                    out=t[:, :n * SWIN],
                    in_=st_d[:, b * STB * SWIN:(b * STB + n) * SWIN],
                )
                st_tiles[b] = t
                st_issued[0] = b + 1

            issue_st()
            if NSTB > 1:
                issue_st()

            def do_visit(g, ci, cc, cl, ps, first, last, vcol):
                tile_h = call_tiles[g][ci]
                jj = cl - calls[g][ci][0]
                b, o = divmod(vcol, STB)
                while st_issued[0] <= min(b + 4, NSTB - 1):
                    issue_st()
                stt = st_tiles[b]
                nc.tensor.matmul(
                    out=ps[:],
                    lhsT=tile_h[:, jj, 0:64],
                    rhs=stt[:, o * SWIN:(o + 1) * SWIN],
                    start=first,
                    stop=last,
                )

            vcol = 0
            j = 0
            for w in range(NW):
                wn = min(WIN, TPC - w * WIN)
                nsw = (wn + SWIN - 1) // SWIN
                ys = [None, None]
                for L in range(2):
                    yL = ysbp.tile([C, WIN], bf16, tag=f"y{L}")
                    for sw in range(nsw):
                        _, _, _, t0, swn = BINS[j]
                        nvis = sum(hi - lo + 1 for (_, lo, hi) in VIS[j])
                        ps = psp.tile([C, SWIN], f32, tag="ps")
                        t_i = 0
                        for (g, lo, hi) in VIS[j]:
                            for cc in range(lo, hi + 1):
                                cl = cc - GBASE[g]
                                ci = cl // GC
                                while issued[g] <= min(ci + 2, len(calls[g]) - 1):
                                    issue_call(g)
                                do_visit(g, ci, cc, cl, ps,
                                         t_i == 0, t_i == nvis - 1, vcol)
                                t_i += 1
                                vcol += 1
                        nc.scalar.copy(out=yL[:, sw * SWIN:sw * SWIN + swn],
                                       in_=ps[:, :swn])
                        j += 1
                    ys[L] = yL

                # ---- dense epilogue for window w ----
                xTw = xtp.tile([C, WIN], bf16, tag="xt")
                nc.sync.dma_start(out=xTw[:, :wn], in_=xT[:, w * WIN:w * WIN + wn])
                r = rpp.tile([C, WIN], f32, tag="r")
                nc.tensor.matmul(out=r[:, :wn], lhsT=wts_t[:, 0:C], rhs=ys[0][:, :wn], start=True, stop=False)
                nc.tensor.matmul(out=r[:, :wn], lhsT=wts_t[:, C:2 * C], rhs=ys[1][:, :wn], start=False, stop=False)
                nc.tensor.matmul(out=r[:, :wn], lhsT=wts_t[:, 2 * C:3 * C], rhs=xTw[:, :wn], start=False, stop=True)
                # s = sigmoid(r) = 0.5*tanh(r/2) + 0.5; store t = tanh(r/2) and
                # fold the affine into the att matmul + final gate multiply, so
                # every ACT func (Tanh/Relu/Exp/Copy) lives in one table set.
                s_sb = ssbp.tile([C + 1, WIN], f32, tag="s_sb")
                nc.scalar.activation(out=s_sb[0:C, :wn], in_=r[:, :wn], func=mybir.ActivationFunctionType.Tanh, scale=0.5)
                g_ps = gpp.tile([1, WIN], f32, tag="g")
                nc.tensor.matmul(out=g_ps[:, :wn], lhsT=att_t[:], rhs=s_sb[0:C, :wn], start=True, stop=True)
                # g = att^T s = 0.5*g' + catt; elu(g) = relu(g) + exp(-relu(-g)) - 1
                t1 = epool.tile([1, WIN], f32, tag="t1")
                t2 = epool.tile([1, WIN], f32, tag="t2")
                nc.scalar.activation(out=t1[:, :wn], in_=g_ps[:, :wn], func=mybir.ActivationFunctionType.Relu, scale=0.5, bias=cb_t[:, 0:1])
                nc.scalar.activation(out=t2[:, :wn], in_=g_ps[:, :wn], func=mybir.ActivationFunctionType.Relu, scale=-0.5, bias=cb_t[:, 1:2])
                nc.scalar.activation(out=t2[:, :wn], in_=t2[:, :wn], func=mybir.ActivationFunctionType.Exp, scale=-1.0)
                nc.vector.scalar_tensor_tensor(out=s_sb[C:C + 1, :wn], in0=t1[:, :wn], scalar=-1.0, in1=t2[:, :wn], op0=mybir.AluOpType.add, op1=mybir.AluOpType.add)

                # ---- transpose + gate + store ----
                for b in range((wn + 127) // 128):
                    r0 = w * WIN + b * 128
                    rn = min(128, TPC - r0)
                    pt = tpp.tile([128, C + 1], f32, tag="pt")
                    nc.tensor.transpose(
                        out=pt[:rn, :],
                        in_=s_sb[:, b * 128:b * 128 + rn],
                        identity=ident_t[:C + 1, :C + 1],
                    )
                    # out = s*elu = (0.5 t + 0.5)*gate = t*g2 + g2, g2 = gate/2
                    g2 = epool.tile([128, 1], f32, tag="gate")
                    nc.scalar.activation(out=g2[:rn, :], in_=pt[:rn, C:C + 1], func=mybir.ActivationFunctionType.Copy, scale=0.5)
                    ot = epool.tile([128, C], f32, tag="ot")
                    nc.vector.tensor_scalar(
                        out=ot[:rn, :],
                        in0=pt[:rn, 0:C],
                        scalar1=g2[:rn, :],
                        scalar2=g2[:rn, :],
                        op0=mybir.AluOpType.mult,
                        op1=mybir.AluOpType.add,
                    )
                    nc.sync.dma_start(out=out_d[r0:r0 + rn, :], in_=ot[:rn, :])
    nc.compile()
    return nc


def _plan(pre):
    """Build segment-major streams + visit ranges. Returns plan dict."""
    # per core, per segment: concatenated bin-ordered messages
    cum = np.zeros((NCORES, 4, NB + 1), np.int64)
    binmsg = []  # [k][g][j] = (src_rel, val, rel_in_subwindow)
    for k in range(NCORES):
        per_g = [[] for _ in range(4)]
        for (w, L, sw, t0, swn) in BINS:
            s, v, tl = pre[L][k]
            lo, hi = np.searchsorted(tl, [t0, t0 + swn])
            sm, vm, tm = s[lo:hi], v[lo:hi], tl[lo:hi] - t0
            gsel = sm // NSEG
            for g in range(4):
                m = gsel == g
                per_g[g].append((sm[m] - g * NSEG, vm[m], tm[m]))
        for g in range(4):
            off = 0
            for j in range(NB):
                cum[k][g][j] = off
                off += len(per_g[g][j][0])
            cum[k][g][NB] = off
        binmsg.append(per_g)
    # drift anchors: every ANCHOR bins all cores pad to the shared cross-core
    # max (equality, not alignment, is what resets drift), so visit ranges
    # stop inflating with cumulative cross-core count divergence
    for g in range(4):
        for a in range(ANCHOR, NB, ANCHOR):
            tgt = int(cum[:, g, a].max())
            delta = tgt - cum[:, g, a]
            cum[:, g, a:] += delta[:, None]
    SCH = [int(max((cum[k][g][NB] + 127) // 128 for k in range(NCORES))) for g in range(4)]
    GBASE = [0, 0, 0, 0]
    for g in range(1, 4):
        GBASE[g] = GBASE[g - 1] + SCH[g - 1]
    NCH = sum(SCH)
    VIS = []
    for j in range(NB):
        vj = []
        for g in range(4):
            lo = min(int(cum[k][g][j]) // 128 for k in range(NCORES))
            hi = max((int(cum[k][g][j + 1]) - 1) // 128 for k in range(NCORES))
            hi = max(hi, lo)
            if j == NB - 1:
                hi = SCH[g] - 1
            vj.append((g, GBASE[g] + lo, GBASE[g] + hi))
        VIS.append(vj)
    NV = int(sum(hi - lo + 1 for vj in VIS for (_, lo, hi) in vj))
    return dict(cum=cum, binmsg=binmsg, SCH=SCH, GBASE=GBASE, NCH=NCH, VIS=VIS, NV=NV)


def _pack16(idx_flat):
    """[S] int16 -> [128, S/16] (16-row wrap, replicated x8)."""
    a = np.asarray(idx_flat, np.int16).reshape(-1, 16).T
    return np.tile(a, (8, 1)).copy()


def kernel(x_1, down_indices, down_values, up_indices, up_values,
           W_down, W_up, W_id, att_down, att_up, att_layer):
    global LAST_EXEC_NS
    x_1 = np.ascontiguousarray(np.asarray(x_1, np.float32))

    pre = [_preprocess(down_indices, down_values), _preprocess(up_indices, up_values)]
    plan = _plan(pre)
    SCH, GBASE, NCH, VIS, NV = plan['SCH'], plan['GBASE'], plan['NCH'], plan['VIS'], plan['NV']
    cum, binmsg = plan['cum'], plan['binmsg']

    xp = np.zeros((N, 128), np_bf16)
    xp[:, :C] = x_1.astype(np_bf16)
    x16 = np.ascontiguousarray(xp[:, :C])
    ident = np.eye(128, dtype=np.float32)
    wts = np.concatenate(
        [np.asarray(W_down, np.float32), np.asarray(W_up, np.float32),
         (1.0 + EPS) * np.asarray(W_id, np.float32)],
        axis=1,
    ).astype(np_bf16)
    att32 = np.asarray(att_layer, np.float32)
    catt = 0.5 * float(att32.sum())
    cb = np.array([[catt, -catt]], np.float32)

    rows128 = np.arange(128)
    in_maps = []
    for k in range(NCORES):
        idx = np.zeros(NCH * 128, np.int16)
        val = np.zeros(NCH * 128, np.float32)
        tlg = np.full(NCH * 128, -1, np.int64)
        binof = np.full(NCH * 128, -1, np.int64)
        for g in range(4):
            for j in range(NB):
                sw_, vw_, tw_ = binmsg[k][g][j]
                a = GBASE[g] * 128 + int(cum[k][g][j])
                n = len(sw_)
                idx[a:a + n] = sw_
                val[a:a + n] = vw_
                tlg[a:a + n] = tw_
                binof[a:a + n] = j
        # precomputed selection matrices: one [128, SWIN] block per visit
        st = np.zeros((128, NV * SWIN), np_bf16)
        vc = 0
        for j in range(NB):
            for (g, lo, hi) in VIS[j]:
                for cc in range(lo, hi + 1):
                    sl = slice(cc * 128, cc * 128 + 128)
                    m = binof[sl] == j
                    st[rows128[m], vc * SWIN + tlg[sl][m]] = val[sl][m]
                    vc += 1
        m = {
            "xp": xp,
            "xT": np.ascontiguousarray(x16[k * TPC:(k + 1) * TPC].T),
            "idx": _pack16(idx),
            "st": st,
            "wts": wts, "att": att32, "cb": cb, "ident": ident,
        }
        in_maps.append(m)

    key = (tuple(SCH), tuple(tuple(v) for vj in VIS for v in vj))
    if key not in _frontend_cache:
        _frontend_cache.clear()
        _frontend_cache[key] = _build_program(SCH, GBASE, NCH, VIS, NV)
    nc = _frontend_cache[key]

    res = run_bass_kernel_spmd(nc, in_maps, core_ids=list(range(NCORES)), trace=True)
    LAST_EXEC_NS = res.exec_time_ns
    out = np.concatenate([res.results[k]["out"] for k in range(NCORES)], axis=0)
    return out.astype(np.float32)


# revision 24
# speedup vs baseline: 1.1987x; 1.1987x over previous
"""Trainium2 Bass kernel for nn_CANLayer (gnn_message_passing) — v6.

Math: softmax over a singleton axis makes attention weights identically 1.0,
so each conv is a plain sparse matmul:
    out = sigmoid(A_d @ x @ Wd + A_u @ x @ Wu + (1+eps) x @ Wi); out *= elu(out @ a)

v3: gathers via the Ant dma_gather ucode (int16 indices); x is zero-padded to
256B rows (dma_gather elem constraint); indices split into 4 row segments of
25000 (int16 range); messages form 4 segment-major streams ordered by bin,
chunks of 128 slots, mid-stream pads use idx=0/val=0. A bin's messages span a
chunk range per segment stream; straddle chunks are matmul'd once per bin.

v4: 4 SWDGE queues — dma_gather queue_num=g pins segment stream g's
descriptor generation to Q7 core pair g, 4-way parallel (the ucode gates on
cpu_id/2 == queue_num; 3.1x on a gather microbench).

v5: bins narrowed to 128-target subwindows (4 bins per lap per 512 epilogue
window); per-subwindow PSUM [C,128] accumulators are copied into the [C,512]
ys tile, keeping the fused 512-wide epilogue; 16-bit path moved to bf16.

v6: DVE was still the wall (is_equal selection-matrix gen has a ~330-cycle
per-instruction floor). The [128,128] selection matrices (val folded in) are
now PRECOMPUTED on host and streamed from DRAM in STB-visit blocks (~88 MB
per core — HBM had headroom), so the scatter path on-device is matmul-only.

v7: epilogue rewritten to stay in ONE ACT table set (sigmoid/exp alternation
cost 2x ~2.7us table reloads per window): sigmoid(r) = 0.5*tanh(r/2)+0.5
with the affine folded into the att matmul constant and the final gate
multiply; elu(g) = relu(g) + exp(-relu(-g)) - 1 via Relu/Exp scale/bias
folding. Gather calls grown past the 64-descriptor single-packet limit with
single_packet=False (GC=21 chunks/call, 4 bufs), per-queue idx tiles so each
queue's first gather starts immediately, and the st stream double-buffered
48 visits/block 2 blocks ahead. Measured 591-718 us across runs (device-load noise ~10%);
baseline was 2327 us. fp8/int8 st were tried and rejected (fp8 rel err
3.3e-2 > 2e-2 gate; int8 rhs not a valid PE dtype in bass).
"""
import numpy as np
import ml_dtypes
from contextlib import ExitStack

import concourse.bacc as bacc
import concourse.bass as bass
import concourse.mybir as mybir
import concourse.tile as tile
from concourse.bass_utils import run_bass_kernel_spmd


def _install_ntff_shim():
    """Best-effort: some runtimes lack antenv.axon_hooks, which
    run_bass_kernel_spmd(trace=True) imports under axon."""
    import sys, types
    try:
        import antenv.axon_hooks  # noqa: F401
        return
    except ImportError:
        pass
    try:
        from trn_agent_boot.trn_boot import _ntff_profile_via_ctypes
        hook = _ntff_profile_via_ctypes('/opt/axon/libaxon_pjrt.so')
        import antenv
        mod = types.ModuleType('antenv.axon_hooks')
        mod.get_axon_ntff_profile_hook = lambda: hook
        mod.set_axon_ntff_profile_hook = lambda h: None
        antenv.axon_hooks = mod
        sys.modules['antenv.axon_hooks'] = mod
    except Exception:
        pass


_install_ntff_shim()

N = 100000
C = 64
NCORES = 8
TPC = 12500
WIN = 512                      # epilogue window
SWIN = 128                     # scatter subwindow (st width)
NW = (TPC + WIN - 1) // WIN    # 25
NSEG = 25000                   # int16 index range per gather segment
GC = 21                        # chunks per dma_gather call (2688 idxs; multi-packet)
STB = 48                       # visits per streamed st block
EPS = 1e-5
ANCHOR = 4                     # drift anchor every ANCHOR bins

LAST_EXEC_NS = None

_frontend_cache = {}

f32 = mybir.dt.float32
bf16 = mybir.dt.bfloat16
i16 = mybir.dt.int16
np_bf16 = ml_dtypes.bfloat16


def _bins():
    """Stream-ordered bin list: (w, L, sw, t0, swn) with t0 core-relative."""
    out = []
    for w in range(NW):
        wn = min(WIN, TPC - w * WIN)
        nsw = (wn + SWIN - 1) // SWIN
        for L in range(2):
            for sw in range(nsw):
                t0 = w * WIN + sw * SWIN
                swn = min(SWIN, wn - sw * SWIN)
                out.append((w, L, sw, t0, swn))
    return out


BINS = _bins()
NB = len(BINS)


def _preprocess(indices, values):
    """Per core: tgt-sorted stream (src, val, tl) for one Laplacian."""
    tgt = np.asarray(indices[0], np.int64)
    src = np.asarray(indices[1], np.int64)
    val = np.asarray(values, np.float32)
    out = []
    for k in range(NCORES):
        base = k * TPC
        sel = (tgt >= base) & (tgt < base + TPC)
        tl = tgt[sel] - base
        s = src[sel]
        v = val[sel]
        order = np.argsort(tl, kind="stable")
        out.append((s[order], v[order], tl[order]))
    return out


def _build_program(SCH, GBASE, NCH, VIS, NV):
    """SCH[g]=chunks per segment stream; VIS[j]=[(g, lo, hi)] visit ranges
    (global chunk ids) for bin j (BINS order); NV total visit columns."""
    nc = bacc.Bacc("TRN2", target_bir_lowering=False, debug=False,
                   num_swdge_queues=4)

    NSTB = (NV + STB - 1) // STB
    xp = nc.dram_tensor("xp", [N, 128], bf16, kind="ExternalInput")
    xT = nc.dram_tensor("xT", [C, TPC], bf16, kind="ExternalInput")
    idx_d = nc.dram_tensor("idx", [128, NCH * 8], i16, kind="ExternalInput")
    st_d = nc.dram_tensor("st", [128, NV * SWIN], bf16, kind="ExternalInput")
    wts_d = nc.dram_tensor("wts", [C, 3 * C], bf16, kind="ExternalInput")  # Wd|Wu|Wi'
    att_d = nc.dram_tensor("att", [C, 1], f32, kind="ExternalInput")
    cb_d = nc.dram_tensor("cb", [1, 2], f32, kind="ExternalInput")  # [catt, -catt]
    ident_d = nc.dram_tensor("ident", [128, 128], f32, kind="ExternalInput")
    out_d = nc.dram_tensor("out", [TPC, C], f32, kind="ExternalOutput")

    with tile.TileContext(nc) as tc, ExitStack() as _stk:
        if True:
            constp = _stk.enter_context(tc.tile_pool(name="const", bufs=1))
            metap = _stk.enter_context(tc.tile_pool(name="meta", bufs=1))
            mp0 = _stk.enter_context(tc.tile_pool(name="m0", bufs=4))
            mp1 = _stk.enter_context(tc.tile_pool(name="m1", bufs=4))
            mp2 = _stk.enter_context(tc.tile_pool(name="m2", bufs=4))
            mp3 = _stk.enter_context(tc.tile_pool(name="m3", bufs=4))
            stbp = _stk.enter_context(tc.tile_pool(name="stb", bufs=4))
            ysbp = _stk.enter_context(tc.tile_pool(name="ysb", bufs=4))
            xtp = _stk.enter_context(tc.tile_pool(name="xt", bufs=2))
            ssbp = _stk.enter_context(tc.tile_pool(name="ssb", bufs=2))
            epool = _stk.enter_context(tc.tile_pool(name="ep", bufs=8))
            psp = _stk.enter_context(tc.tile_pool(name="ps", bufs=3, space="PSUM"))
            rpp = _stk.enter_context(tc.tile_pool(name="rp", bufs=2, space="PSUM"))
            gpp = _stk.enter_context(tc.tile_pool(name="gp", bufs=1, space="PSUM"))
            tpp = _stk.enter_context(tc.tile_pool(name="tp", bufs=2, space="PSUM"))
            mpools = [mp0, mp1, mp2, mp3]
            ident_t = constp.tile([128, 128], f32)
            nc.sync.dma_start(out=ident_t[:], in_=ident_d[:])
            wts_t = constp.tile([C, 3 * C], bf16)
            nc.sync.dma_start(out=wts_t[:], in_=wts_d[:])
            att_t = constp.tile([C, 1], f32)
            nc.sync.dma_start(out=att_t[:], in_=att_d[:])
            cb_t = constp.tile([1, 2], f32)
            nc.sync.dma_start(out=cb_t[:], in_=cb_d[:])

            idx_ts = []
            for g in range(4):
                tg = metap.tile([128, SCH[g] * 8], i16, name=f"idx_t{g}")
                nc.sync.dma_start(
                    out=tg[:],
                    in_=idx_d[:, GBASE[g] * 8:(GBASE[g] + SCH[g]) * 8],
                )
                idx_ts.append(tg)

            # call schedule per segment stream: groups of GC chunks
            calls = []            # [g] -> list of (startchunk_local, nchunks)
            for g in range(4):
                cl = []
                a = 0
                while a < SCH[g]:
                    cl.append((a, min(GC, SCH[g] - a)))
                    a += min(GC, SCH[g] - a)
                calls.append(cl)
            call_tiles = [dict() for _ in range(4)]
            issued = [0, 0, 0, 0]

            def issue_call(g):
                ci = issued[g]
                a, cnt = calls[g][ci]
                t = mpools[g].tile([128, GC, 128], bf16, tag=f"m{g}")
                ni = cnt * 128
                io = a * 8
                nc.gpsimd.dma_gather(
                    t[:, :cnt, :],
                    xp[g * NSEG:(g + 1) * NSEG, :],
                    idx_ts[g][:, io:io + cnt * 8],
                    ni, ni, 128, elem_step=128, queue_num=g,
                    single_packet=False,
                )
                call_tiles[g][ci] = t
                issued[g] = ci + 1

            for g in range(4):
                issue_call(g)
                if len(calls[g]) > 1:
                    issue_call(g)

            # st stream: blocks of STB visit columns, double-buffered
            st_tiles = {}
            st_issued = [0]

            def issue_st():
                b = st_issued[0]
                t = stbp.tile([128, STB * SWIN], bf16, tag="stb")
                n = min(STB, NV - b * STB)
                nc.sync.dma_start(
                    out=t[:, :n * SWIN],
                    in_=st_d[:, b * STB * SWIN:(b * STB + n) * SWIN],
                )
                st_tiles[b] = t
                st_issued[0] = b + 1

            issue_st()
            if NSTB > 1:
                issue_st()

            def do_visit(g, ci, cc, cl, ps, first, last, vcol):
                tile_h = call_tiles[g][ci]
                jj = cl - calls[g][ci][0]
                b, o = divmod(vcol, STB)
                while st_issued[0] <= min(b + 2, NSTB - 1):
                    issue_st()
                stt = st_tiles[b]
                nc.tensor.matmul(
                    out=ps[:],
                    lhsT=tile_h[:, jj, 0:64],
                    rhs=stt[:, o * SWIN:(o + 1) * SWIN],
                    start=first,
                    stop=last,
                )

            vcol = 0
            j = 0
            for w in range(NW):
                wn = min(WIN, TPC - w * WIN)
                nsw = (wn + SWIN - 1) // SWIN
                ys = [None, None]
                for L in range(2):
                    yL = ysbp.tile([C, WIN], bf16, tag=f"y{L}")
                    for sw in range(nsw):
                        _, _, _, t0, swn = BINS[j]
                        nvis = sum(hi - lo + 1 for (_, lo, hi) in VIS[j])
                        ps = psp.tile([C, SWIN], f32, tag="ps")
                        t_i = 0
                        for (g, lo, hi) in VIS[j]:
                            for cc in range(lo, hi + 1):
                                cl = cc - GBASE[g]
                                ci = cl // GC
                                while issued[g] <= min(ci + 1, len(calls[g]) - 1):
                                    issue_call(g)
                                do_visit(g, ci, cc, cl, ps,
                                         t_i == 0, t_i == nvis - 1, vcol)
                                t_i += 1
                                vcol += 1
                        nc.scalar.copy(out=yL[:, sw * SWIN:sw * SWIN + swn],
                                       in_=ps[:, :swn])
                        j += 1
                    ys[L] = yL

                # ---- dense epilogue for window w ----
                xTw = xtp.tile([C, WIN], bf16, tag="xt")
                nc.sync.dma_start(out=xTw[:, :wn], in_=xT[:, w * WIN:w * WIN + wn])
                r = rpp.tile([C, WIN], f32, tag="r")
                nc.tensor.matmul(out=r[:, :wn], lhsT=wts_t[:, 0:C], rhs=ys[0][:, :wn], start=True, stop=False)
                nc.tensor.matmul(out=r[:, :wn], lhsT=wts_t[:, C:2 * C], rhs=ys[1][:, :wn], start=False, stop=False)
                nc.tensor.matmul(out=r[:, :wn], lhsT=wts_t[:, 2 * C:3 * C], rhs=xTw[:, :wn], start=False, stop=True)
                # s = sigmoid(r) = 0.5*tanh(r/2) + 0.5; store t = tanh(r/2) and
                # fold the affine into the att matmul + final gate multiply, so
                # every ACT func (Tanh/Relu/Exp/Copy) lives in one table set.
                s_sb = ssbp.tile([C + 1, WIN], f32, tag="s_sb")
                nc.scalar.activation(out=s_sb[0:C, :wn], in_=r[:, :wn], func=mybir.ActivationFunctionType.Tanh, scale=0.5)
                g_ps = gpp.tile([1, WIN], f32, tag="g")
                nc.tensor.matmul(out=g_ps[:, :wn], lhsT=att_t[:], rhs=s_sb[0:C, :wn], start=True, stop=True)
                # g = att^T s = 0.5*g' + catt; elu(g) = relu(g) + exp(-relu(-g)) - 1
                t1 = epool.tile([1, WIN], f32, tag="t1")
                t2 = epool.tile([1, WIN], f32, tag="t2")
                nc.scalar.activation(out=t1[:, :wn], in_=g_ps[:, :wn], func=mybir.ActivationFunctionType.Relu, scale=0.5, bias=cb_t[:, 0:1])
                nc.scalar.activation(out=t2[:, :wn], in_=g_ps[:, :wn], func=mybir.ActivationFunctionType.Relu, scale=-0.5, bias=cb_t[:, 1:2])
                nc.scalar.activation(out=t2[:, :wn], in_=t2[:, :wn], func=mybir.ActivationFunctionType.Exp, scale=-1.0)
                nc.vector.scalar_tensor_tensor(out=s_sb[C:C + 1, :wn], in0=t1[:, :wn], scalar=-1.0, in1=t2[:, :wn], op0=mybir.AluOpType.add, op1=mybir.AluOpType.add)

                # ---- transpose + gate + store ----
                for b in range((wn + 127) // 128):
                    r0 = w * WIN + b * 128
                    rn = min(128, TPC - r0)
                    pt = tpp.tile([128, C + 1], f32, tag="pt")
                    nc.tensor.transpose(
                        out=pt[:rn, :],
                        in_=s_sb[:, b * 128:b * 128 + rn],
                        identity=ident_t[:C + 1, :C + 1],
                    )
                    # out = s*elu = (0.5 t + 0.5)*gate = t*g2 + g2, g2 = gate/2
                    g2 = epool.tile([128, 1], f32, tag="gate")
                    nc.scalar.activation(out=g2[:rn, :], in_=pt[:rn, C:C + 1], func=mybir.ActivationFunctionType.Copy, scale=0.5)
                    ot = epool.tile([128, C], f32, tag="ot")
                    nc.vector.tensor_scalar(
                        out=ot[:rn, :],
                        in0=pt[:rn, 0:C],
                        scalar1=g2[:rn, :],
                        scalar2=g2[:rn, :],
                        op0=mybir.AluOpType.mult,
                        op1=mybir.AluOpType.add,
                    )
                    nc.sync.dma_start(out=out_d[r0:r0 + rn, :], in_=ot[:rn, :])
    nc.compile()
    return nc


def _plan(pre):
    """Build segment-major streams + visit ranges. Returns plan dict."""
    # per core, per segment: concatenated bin-ordered messages
    cum = np.zeros((NCORES, 4, NB + 1), np.int64)
    binmsg = []  # [k][g][j] = (src_rel, val, rel_in_subwindow)
    for k in range(NCORES):
        per_g = [[] for _ in range(4)]
        for (w, L, sw, t0, swn) in BINS:
            s, v, tl = pre[L][k]
            lo, hi = np.searchsorted(tl, [t0, t0 + swn])
            sm, vm, tm = s[lo:hi], v[lo:hi], tl[lo:hi] - t0
            gsel = sm // NSEG
            for g in range(4):
                m = gsel == g
                per_g[g].append((sm[m] - g * NSEG, vm[m], tm[m]))
        for g in range(4):
            off = 0
            for j in range(NB):
                cum[k][g][j] = off
                off += len(per_g[g][j][0])
            cum[k][g][NB] = off
        binmsg.append(per_g)
    # drift anchors: every ANCHOR bins all cores pad to the shared cross-core
    # max (equality, not alignment, is what resets drift), so visit ranges
    # stop inflating with cumulative cross-core count divergence
    for g in range(4):
        for a in range(ANCHOR, NB, ANCHOR):
            tgt = int(cum[:, g, a].max())
            delta = tgt - cum[:, g, a]
            cum[:, g, a:] += delta[:, None]
    SCH = [int(max((cum[k][g][NB] + 127) // 128 for k in range(NCORES))) for g in range(4)]
    GBASE = [0, 0, 0, 0]
    for g in range(1, 4):
        GBASE[g] = GBASE[g - 1] + SCH[g - 1]
    NCH = sum(SCH)
    VIS = []
    for j in range(NB):
        vj = []
        for g in range(4):
            lo = min(int(cum[k][g][j]) // 128 for k in range(NCORES))
            hi = max((int(cum[k][g][j + 1]) - 1) // 128 for k in range(NCORES))
            hi = max(hi, lo)
            if j == NB - 1:
                hi = SCH[g] - 1
            vj.append((g, GBASE[g] + lo, GBASE[g] + hi))
        VIS.append(vj)
    NV = int(sum(hi - lo + 1 for vj in VIS for (_, lo, hi) in vj))
    return dict(cum=cum, binmsg=binmsg, SCH=SCH, GBASE=GBASE, NCH=NCH, VIS=VIS, NV=NV)


def _pack16(idx_flat):
    """[S] int16 -> [128, S/16] (16-row wrap, replicated x8)."""
    a = np.asarray(idx_flat, np.int16).reshape(-1, 16).T
    return np.tile(a, (8, 1)).copy()


def kernel(x_1, down_indices, down_values, up_indices, up_values,
           W_down, W_up, W_id, att_down, att_up, att_layer):
    global LAST_EXEC_NS
    x_1 = np.ascontiguousarray(np.asarray(x_1, np.float32))

    pre = [_preprocess(down_indices, down_values), _preprocess(up_indices, up_values)]
    plan = _plan(pre)
    SCH, GBASE, NCH, VIS, NV = plan['SCH'], plan['GBASE'], plan['NCH'], plan['VIS'], plan['NV']
    cum, binmsg = plan['cum'], plan['binmsg']

    xp = np.zeros((N, 128), np_bf16)
    xp[:, :C] = x_1.astype(np_bf16)
    x16 = np.ascontiguousarray(xp[:, :C])
    ident = np.eye(128, dtype=np.float32)
    wts = np.concatenate(
        [np.asarray(W_down, np.float32), np.asarray(W_up, np.float32),
         (1.0 + EPS) * np.asarray(W_id, np.float32)],
        axis=1,
    ).astype(np_bf16)
    att32 = np.asarray(att_layer, np.float32)
    catt = 0.5 * float(att32.sum())
    cb = np.array([[catt, -catt]], np.float32)

    rows128 = np.arange(128)
    in_maps = []
    for k in range(NCORES):
        idx = np.zeros(NCH * 128, np.int16)
        val = np.zeros(NCH * 128, np.float32)
        tlg = np.full(NCH * 128, -1, np.int64)
        binof = np.full(NCH * 128, -1, np.int64)
        for g in range(4):
            for j in range(NB):
                sw_, vw_, tw_ = binmsg[k][g][j]
                a = GBASE[g] * 128 + int(cum[k][g][j])
                n = len(sw_)
                idx[a:a + n] = sw_
                val[a:a + n] = vw_
                tlg[a:a + n] = tw_
                binof[a:a + n] = j
        # precomputed selection matrices: one [128, SWIN] block per visit
        st = np.zeros((128, NV * SWIN), np_bf16)
        vc = 0
        for j in range(NB):
            for (g, lo, hi) in VIS[j]:
                for cc in range(lo, hi + 1):
                    sl = slice(cc * 128, cc * 128 + 128)
                    m = binof[sl] == j
                    st[rows128[m], vc * SWIN + tlg[sl][m]] = val[sl][m]
                    vc += 1
        m = {
            "xp": xp,
            "xT": np.ascontiguousarray(x16[k * TPC:(k + 1) * TPC].T),
            "idx": _pack16(idx),
            "st": st,
            "wts": wts, "att": att32, "cb": cb, "ident": ident,
        }
        in_maps.append(m)

    key = (tuple(SCH), tuple(tuple(v) for vj in VIS for v in vj))
    if key not in _frontend_cache:
        _frontend_cache.clear()
        _frontend_cache[key] = _build_program(SCH, GBASE, NCH, VIS, NV)
    nc = _frontend_cache[key]

    res = run_bass_kernel_spmd(nc, in_maps, core_ids=list(range(NCORES)), trace=True)
    LAST_EXEC_NS = res.exec_time_ns
    out = np.concatenate([res.results[k]["out"] for k in range(NCORES)], axis=0)
    return out.astype(np.float32)
